# revision 32
# baseline (speedup 1.0000x reference)
"""Trainium2 Bass kernel for a GCN layer (gnn_message_passing).

Reference computation (per batch b):
    node_msg = h @ Wn_w.T + Wn_b                  # (N, OD)
    edge_msg = edge_feat @ We_w.T + We_b          # (N, N, OD)
    self_msg = h @ Ws_w.T + Ws_b                  # (N, OD)
    deg      = adj.sum(-1)                        # (N,)
    agg      = node_msg * deg + einsum('ij,ijo->io', adj, edge_msg)
    out      = relu(agg / clip(deg, 1) + self_msg)

Key algebraic rewrite: the (N,N,OD) edge_msg is never materialized.
    einsum('ij,ijo->io', adj, edge_feat @ We.T + We_b)
      = (einsum('ij,ije->ie', adj, edge_feat)) @ We.T + deg * We_b
so the dominant work is the adj-masked reduction of edge_feat over the
source-node axis j, producing (N, ED), followed by a tiny 16->64 matmul.

Sharding: data-parallel over batch B=8 across the 8 NeuronCores (one
batch element per core); weights replicated.

v2 pipeline design (per core) — three concurrent DMA queues:
  The cost model serializes DMA transfers on the ISSUING engine's queue,
  but different queues' transfers overlap freely.  So the 16 MiB ef
  stream is split by j across the SP HWDGE queue (j < JS) and the Pool
  SWDGE queue (j >= JS), running concurrently; ACT's queue carries the
  small tensors (adj / h / weights / biases / out), all coalesced into
  single multi-block DMAs where possible.

  Masked reduce  msum[i,e] = sum_j ef[i,j,e] * adj[i,j] (raw int32 adj)  by channel:
    e in [0, NA):   DVE scalar_tensor_tensor with accum_out, one instr
                    per (block, j-range).
    e in [NA, 16):  Pool tensor_tensor product (adj broadcast over the
                    channel axis via a stride-0 AP) writing a prod tile,
                    then ACT activation(Copy, accum_out) per channel.
  ACT also does the fused adj widen(int32->f32)+degree (activation Copy
  with accum_out).  Per-block glue: transpose(msum) -> (16->64) matmul
  -> (r*pes)+base -> relu, with base = degr*(h Wn^T + bn + be) + h Ws^T
  + bs precomputed off the critical path.  The last block's SP chunk is
  split so the serial tail after the final DMA is short.
"""

import os
import sys
from contextlib import ExitStack

import numpy as np


def _ensure_concourse():
    try:
        import concourse  # noqa: F401
        return
    except ImportError:
        pass
    for p in ("/opt/trn_rl_repo", "/root/.axon_site/_ro/trn_rl_repo"):
        if os.path.isdir(p) and p not in sys.path:
            sys.path.insert(0, p)
            try:
                import concourse  # noqa: F401
                return
            except ImportError:
                continue
    raise ImportError("cannot locate the concourse (bass) package")


_ensure_concourse()

import concourse.bacc as bacc  # noqa: E402
import concourse.bass as bass  # noqa: E402
import concourse.tile as tile  # noqa: E402
from concourse import mybir  # noqa: E402
from concourse.bass_utils import run_bass_kernel_spmd  # noqa: E402
from concourse.masks import make_identity  # noqa: E402

B, N, ND, ED, OD = 8, 512, 64, 16, 64
NCORES = 8
PB = 128           # destination-node block (SBUF partitions)
NBLK = N // PB     # 4

F32 = mybir.dt.float32
I32 = mybir.dt.int32

# --- tuning knobs ---
JS = 310           # j < JS streams on the SP queue; j >= JS on Pool SWDGE
NA = 10            # channels 0..NA-1 via DVE stt; NA..15 via Pool+ACT
NC = ED - NA
NA3 = 8            # block 3's (tail) DVE/Pool-tree channel split
NC3 = ED - NA3
JS3A = 252         # block 3's SP chunk is split [0,JS3A) + [JS3A,JS) so the
                   # final arriving piece (and its serial compute) is small

AF_COPY = mybir.ActivationFunctionType.Copy
MULT = mybir.AluOpType.mult
ADD = mybir.AluOpType.add
BYPASS = mybir.AluOpType.bypass


def _row_ap(handle, n):
    """View a 1-D DRAM tensor (n,) as a (1, n) AP."""
    ap = handle.ap()
    return bass.AP(tensor=ap.tensor, offset=ap.offset, ap=[[0, 1], [1, n]])


def _view(ap, axes, extra_offset=0):
    """Re-stride an AP (offset in elements)."""
    return bass.AP(tensor=ap.tensor, offset=ap.offset + extra_offset, ap=axes)


def build_bass(repeat=1, mode="full", unroll=1):
    """Build the single-core Bass program (SPMD across 8 cores)."""
    nc = bacc.Bacc(
        "TRN2",
        target_bir_lowering=False,
        debug=False,
        num_devices=NCORES,
    )

    h_d = nc.dram_tensor("h", [N, ND], F32, kind="ExternalInput")
    adj_d = nc.dram_tensor("adj", [N, N], I32, kind="ExternalInput")
    ef_d = nc.dram_tensor("edge_feat", [N, N, ED], F32, kind="ExternalInput")
    wn_d = nc.dram_tensor("Wn_w", [OD, ND], F32, kind="ExternalInput")
    wnb_d = nc.dram_tensor("Wn_b", [OD], F32, kind="ExternalInput")
    we_d = nc.dram_tensor("We_w", [OD, ED], F32, kind="ExternalInput")
    web_d = nc.dram_tensor("We_b", [OD], F32, kind="ExternalInput")
    ws_d = nc.dram_tensor("Ws_w", [OD, ND], F32, kind="ExternalInput")
    wsb_d = nc.dram_tensor("Ws_b", [OD], F32, kind="ExternalInput")
    out_d = nc.dram_tensor("out", [N, OD], F32, kind="ExternalOutput")

    h_ap = h_d.ap()
    adj_ap = adj_d.ap()
    ef_ap = ef_d.ap()
    out_ap = out_d.ap()

    with tile.TileContext(nc) as tc, ExitStack() as ctx:
        consts = ctx.enter_context(tc.tile_pool(name="consts", bufs=1))
        efp = ctx.enter_context(tc.tile_pool(name="efp", bufs=1))
        adjp = ctx.enter_context(tc.tile_pool(name="adjp", bufs=1))
        work = ctx.enter_context(tc.tile_pool(name="work", bufs=1))
        prodp = ctx.enter_context(tc.tile_pool(name="prodp", bufs=2))
        pset = ctx.enter_context(tc.tile_pool(name="pset", bufs=2, space="PSUM"))
        pmm = ctx.enter_context(tc.tile_pool(name="pmm", bufs=2, space="PSUM"))
        pep = ctx.enter_context(tc.tile_pool(name="pep", bufs=2, space="PSUM"))

        def emit_body():
            ident = consts.tile([128, 128], F32)
            make_identity(nc, ident)

            # ---- ACT queue head: adj first (its widen gates the whole
            # reduce pipeline), then weights/h/biases ----
            # adj: block 0's rows first (gates the first stt piece), then
            # blocks 1-3 coalesced (partition p holds rows {128b + p}).
            adj_t = adjp.tile([PB, NBLK, N], I32, tag="adjt")
            nc.scalar.dma_start(
                out=adj_t[:, 0, :],
                in_=_view(adj_ap, [[N, PB], [1, N]]),
            )
            nc.scalar.dma_start(
                out=adj_t[:, 1:NBLK, :],
                in_=_view(adj_ap, [[N, PB], [N * PB, NBLK - 1], [1, N]],
                          extra_offset=N * PB),
            )

            # weights + h + biases on ACT
            wn_sb = consts.tile([OD, ND], F32, tag="wload")
            nc.scalar.dma_start(out=wn_sb, in_=wn_d.ap())
            ws_sb = consts.tile([OD, ND], F32, tag="wload2")
            nc.scalar.dma_start(out=ws_sb, in_=ws_d.ap())
            we_sb = consts.tile([OD, ED], F32, tag="wload3")
            nc.scalar.dma_start(out=we_sb, in_=we_d.ap())
            h_sb = work.tile([PB, NBLK, ND], F32, tag="hload")
            nc.scalar.dma_start(
                out=h_sb,
                in_=_view(h_ap, [[ND, PB], [ND * PB, NBLK], [1, ND]]),
            )
            bias_n = consts.tile([1, OD], F32)
            nc.scalar.dma_start(out=bias_n, in_=_row_ap(wnb_d, OD))
            bias_e = consts.tile([1, OD], F32)
            nc.scalar.dma_start(out=bias_e, in_=_row_ap(web_d, OD))

            # degree per block: ACT activation(Copy, accum) reading the raw
            # int32 adj (the reduce channels consume int32 adj directly, so
            # no widened copy is needed and this pass is off the critical
            # path -- deg only feeds base/r, used mid-stream).
            deg_scr = work.tile([PB, N], F32, tag="degscr")
            degs = []
            for ib in range(NBLK):
                deg = work.tile([PB, 1], F32, tag=f"deg{ib}")
                nc.scalar.activation(
                    out=deg_scr,
                    in_=adj_t[:, ib, :],
                    func=AF_COPY,
                    accum_out=deg,
                )
                degs.append(deg)

            # ---- ef stream: SP gets j<JS per block; Pool SWDGE the rest.
            # SP's DMAs all go up front (its queue has nothing else); Pool's
            # queue interleaves its ef DMAs with the product work so the
            # products don't all queue behind the stream.  Block 3's SP
            # chunk is split so the final arriving piece is small.
            ef_ts = [
                efp.tile([PB, N, ED], F32, tag=f"ef{ib}", name=f"ef_t{ib}")
                for ib in range(NBLK)
            ]
            sp_pieces = {0: [(0, JS)], 1: [(0, JS)], 2: [(0, JS)],
                         3: [(0, JS3A), (JS3A, JS - JS3A)]}
            for ib in range(NBLK):
                i0 = ib * PB
                for (j0, jn) in sp_pieces[ib]:
                    nc.sync.dma_start(
                        out=ef_ts[ib][:, j0 : j0 + jn, :],
                        in_=ef_ap[i0 : i0 + PB, j0 : j0 + jn, :],
                    )

            def pool_ef(ib, j0=JS, jn=N - JS):
                i0 = ib * PB
                nc.gpsimd.dma_start(
                    out=ef_ts[ib][:, j0 : j0 + jn, :],
                    in_=ef_ap[i0 : i0 + PB, j0 : j0 + jn, :],
                )

            prods = {}   # ib -> prod tile [PB, nc_b, N]

            def pool_prod(ib, j0, jn):
                """Pool products for block ib's ACT channels over [j0, j0+jn)."""
                na_b = NA3 if ib == 3 else NA
                nc_b = ED - na_b
                if ib not in prods:
                    prods[ib] = prodp.tile(
                        [PB, nc_b, N], F32, tag="prod", name=f"prod{ib}"
                    )
                prod = prods[ib]
                ef_apv = ef_ts[ib][:]
                af = adj_t[:, ib, :]
                prod_apv = prod[:]
                in0 = _view(ef_apv, [ef_apv.ap[0], [1, nc_b], [ED, jn]],
                            extra_offset=j0 * ED + na_b)
                in1 = _view(af, [af.ap[0], [0, nc_b], [1, jn]], extra_offset=j0)
                outp_ = _view(prod_apv, [prod_apv.ap[0], [N, nc_b], [1, jn]],
                              extra_offset=j0)
                nc.gpsimd.tensor_tensor(out=outp_, in0=in0, in1=in1, op=MULT)

            # Block 3's ACT channels are reduced by a pairwise-add tree on
            # Pool (idle once its stream ends) instead of ACT activations.
            tree_a = work.tile([PB, NC3, N // 2], F32, tag="treea")
            tree_b = work.tile([PB, NC3, N // 4], F32, tag="treeb")

            def pool_tree(ib):
                prod = prods[ib]
                msum = msums[ib]
                na_b = NA3 if ib == 3 else NA
                nc_b = ED - na_b
                src = prod[:]           # [PB, nc_b, 512]
                src_n = N
                bufs_ = [tree_a, tree_b]
                k = 0
                while src_n > 2:
                    w = src_n // 2
                    dst = bufs_[k % 2]
                    nc.gpsimd.tensor_tensor(
                        out=dst[:, :, 0:w],
                        in0=_view(src, [src.ap[0], [src.ap[1][0], nc_b], [1, w]]),
                        in1=_view(src, [src.ap[0], [src.ap[1][0], nc_b], [1, w]],
                                  extra_offset=w),
                        op=ADD,
                    )
                    src = dst[:]
                    src_n = w
                    k += 1
                # final 2 -> 1 straight into msum columns
                msv = msum[:]
                nc.gpsimd.tensor_tensor(
                    out=_view(msv, [msv.ap[0], [1, nc_b], [1, 1]],
                              extra_offset=na_b),
                    in0=_view(src, [src.ap[0], [src.ap[1][0], nc_b], [1, 1]]),
                    in1=_view(src, [src.ap[0], [src.ap[1][0], nc_b], [1, 1]],
                              extra_offset=1),
                    op=ADD,
                )

            # Pool queue order: two ef DMAs ahead, then alternate.  Block
            # 0's chunk is split so DVE's first stt piece can start ~2.7us.
            pool_ef(0, JS, 64)
            pool_ef(0, JS + 64, N - JS - 64)
            pool_ef(1)
            pool_prod(0, 0, N)
            pool_ef(2)
            pool_prod(1, 0, N)
            pool_ef(3)
            pool_prod(2, 0, N)
            pool_prod(3, 0, JS3A)
            pool_prod(3, JS3A, N - JS3A)
            # pool_tree(3) is emitted in the interleaved section below (no
            # other Pool instructions are emitted in between, so it still
            # lands right after prod3 in Pool's queue).

            # ---- weight prep: transpose on PE; biases folded as extra row --
            rhs_n = consts.tile([ND + 1, OD], F32)
            rhs_s = consts.tile([ND + 1, OD], F32)
            weT = consts.tile([ED, OD], F32)

            pw = pset.tile([ND, OD], F32, tag="t")
            nc.tensor.transpose(pw, wn_sb, ident[:ND, :OD])
            nc.vector.tensor_copy(out=rhs_n[0:ND, :], in_=pw)
            pw2 = pset.tile([ND, OD], F32, tag="t")
            nc.tensor.transpose(pw2, ws_sb, ident[:ND, :OD])
            nc.vector.tensor_copy(out=rhs_s[0:ND, :], in_=pw2)
            pw3 = pset.tile([ED, OD], F32, tag="t")
            nc.tensor.transpose(pw3, we_sb, ident[:ND, :OD])
            nc.vector.tensor_copy(out=weT, in_=pw3)

            nc.vector.tensor_add(rhs_n[ND : ND + 1, :], bias_n, bias_e)
            nc.scalar.dma_start(out=rhs_s[ND : ND + 1, :], in_=_row_ap(wsb_d, OD))

            hT = consts.tile([ND + 1, N], F32)
            nc.vector.memset(hT[ND : ND + 1, :], 1.0)
            rs, degrs, bases = [], [], []

            def emit_mid():
                """Deg chain, h^T and base precompute -- DVE work that
                depends on the ACT widens / h DMA, emitted after stt(0) so
                the DVE queue head never stalls on it."""
                # degc = max(deg,1); r = 1/degc; degr = deg*r
                for ib in range(NBLK):
                    degc = work.tile([PB, 1], F32, tag=f"degc{ib}")
                    nc.vector.tensor_scalar_max(degc, degs[ib], 1.0)
                    r = work.tile([PB, 1], F32, tag=f"r{ib}")
                    nc.vector.reciprocal(r, degc)
                    degr = work.tile([PB, 1], F32, tag=f"degr{ib}")
                    nc.vector.tensor_mul(degr, degs[ib], r)
                    rs.append(r)
                    degrs.append(degr)
                # h^T with an appended ones-row: (65, 512)
                for ib in range(NBLK):
                    ph = pset.tile([ND, PB], F32, tag="t")
                    nc.tensor.transpose(ph, h_sb[:, ib, :], ident)
                    nc.vector.tensor_copy(
                        out=hT[0:ND, ib * PB : (ib + 1) * PB], in_=ph
                    )
                # base = degr*(node+biases) + self
                for ib in range(NBLK):
                    i0 = ib * PB
                    pn = pmm.tile([PB, OD], F32, tag="pn")
                    nc.tensor.matmul(
                        pn, lhsT=hT[:, i0 : i0 + PB], rhs=rhs_n,
                        start=True, stop=True,
                    )
                    hs = pmm.tile([PB, OD], F32, tag="hs")
                    nc.tensor.matmul(
                        hs, lhsT=hT[:, i0 : i0 + PB], rhs=rhs_s,
                        start=True, stop=True,
                    )
                    an = work.tile([PB, OD], F32, tag=f"an{ib}")
                    nc.scalar.activation(
                        out=an, in_=pn, func=AF_COPY, scale=degrs[ib]
                    )
                    base = work.tile([PB, OD], F32, tag=f"base{ib}")
                    nc.vector.tensor_add(base, an, hs)
                    bases.append(base)

            # ---- per-block masked reduce + combine ----
            scr = work.tile([PB, N], F32, tag="scr")       # stt throwaway out
            scr_a = work.tile([PB, N], F32, tag="scra")    # ACT reduce out
            ob = work.tile([PB, NBLK, OD], F32, tag="ob")  # merged output
            msums = {
                ib: work.tile([PB, ED], F32, tag=f"msum{ib}", name=f"msum{ib}")
                for ib in range(NBLK)
            }

            def emit_stt(ib):
                """DVE stt channels -> msum[:, 0:na_b].  Whole-j per channel
                for blocks 0-2; block 3 in two pieces (tail)."""
                ef_t = ef_ts[ib]
                af = adj_t[:, ib, :]
                msum = msums[ib]
                na_b = NA3 if ib == 3 else NA
                if 0 < ib < 3:
                    for e in range(na_b):
                        nc.vector.scalar_tensor_tensor(
                            out=scr[:, 0:N],
                            in0=ef_t[:, :, e],
                            scalar=1.0,
                            in1=af,
                            op0=BYPASS,
                            op1=MULT,
                            accum_out=msum[:, e : e + 1],
                        )
                else:
                    # blocks 0 and 3 reduce in pieces: block 0's Pool chunk
                    # arrives first (split again so DVE starts ~2.7us);
                    # block 3's split shortens the tail.
                    pieces = (
                        ((JS, 64), (JS + 64, N - JS - 64), (0, JS))
                        if ib == 0
                        else ((0, JS3A), (JS3A, N - JS3A))
                    )
                    parts = []
                    for (j0, jn) in pieces:
                        pA = work.tile([PB, na_b], F32, tag=f"pA{ib}_{j0}")
                        for e in range(na_b):
                            nc.vector.scalar_tensor_tensor(
                                out=scr[:, 0:jn],
                                in0=ef_t[:, j0 : j0 + jn, e],
                                scalar=1.0,
                                in1=af[:, j0 : j0 + jn],
                                op0=BYPASS,
                                op1=MULT,
                                accum_out=pA[:, e : e + 1],
                            )
                        parts.append(pA)
                    acc = parts[0]
                    for k in range(1, len(parts) - 1):
                        nxt = work.tile([PB, na_b], F32, tag=f"pacc{ib}_{k}",
                                        name=f"pacc{ib}_{k}")
                        nc.vector.tensor_add(nxt, acc, parts[k])
                        acc = nxt
                    nc.vector.tensor_add(msum[:, 0:na_b], acc, parts[-1])

            def emit_red(ib):
                """ACT activation reduce of Pool products -> msum[:, na_b:]."""
                msum = msums[ib]
                na_b = NA3 if ib == 3 else NA
                nc_b = ED - na_b
                prod = prods[ib]
                if ib < 3:
                    for c in range(nc_b):
                        nc.scalar.activation(
                            out=scr_a[:, 0:N],
                            in_=prod[:, c, :],
                            func=AF_COPY,
                            accum_out=msum[:, na_b + c : na_b + c + 1],
                        )
                else:
                    # block 3: Pool pairwise-add tree instead of ACT
                    pool_tree(ib)

            def emit_glue(ib):
                """(128,16) -> (16,128), project with We^T, combine, relu."""
                msum = msums[ib]
                pm = pset.tile([ED, PB], F32, tag="t")
                nc.tensor.transpose(pm, msum, ident)
                msT = work.tile([ED, PB], F32, tag=f"msT{ib}")
                nc.scalar.copy(out=msT, in_=pm)
                pes = pep.tile([PB, OD], F32, tag="pes")
                nc.tensor.matmul(pes, lhsT=msT, rhs=weT, start=True, stop=True)

                nc.vector.scalar_tensor_tensor(
                    out=ob[:, ib, :],
                    in0=pes,
                    scalar=rs[ib],
                    in1=bases[ib],
                    op0=MULT,
                    op1=ADD,
                )
                nc.vector.tensor_scalar_max(ob[:, ib, :], ob[:, ib, :], 0.0)

            # software-pipelined emission: stt b+1 ahead of glue b so DVE
            # never stalls on a glue chain waiting for ACT.
            emit_stt(0)
            emit_mid()
            emit_red(0)
            emit_stt(1)
            emit_glue(0)
            emit_red(1)
            emit_stt(2)
            emit_glue(1)
            emit_red(2)
            emit_stt(3)
            emit_glue(2)
            emit_red(3)
            emit_glue(3)

            # ---- out DMAs on the idle SP queue: blocks 0-2 as soon as
            # their relu lands, block 3's 32KB alone on the tail ----
            nc.sync.dma_start(
                out=_view(out_ap, [[OD, PB], [OD * PB, NBLK - 1], [1, OD]]),
                in_=ob[:, 0 : NBLK - 1, :],
            )
            nc.sync.dma_start(
                out=_view(out_ap, [[OD, PB], [1, OD]],
                          extra_offset=OD * PB * (NBLK - 1)),
                in_=ob[:, NBLK - 1, :],
            )

        if repeat == 1:
            for _ in range(unroll):
                emit_body()
        else:
            with tc.For_i(0, repeat, 1):
                for _ in range(unroll):
                    emit_body()

    nc.compile()
    return nc


_NC_CACHE = None


def _get_nc():
    global _NC_CACHE
    if _NC_CACHE is None:
        _NC_CACHE = build_bass()
    return _NC_CACHE


def make_in_maps(inputs):
    w = {
        k: np.ascontiguousarray(np.asarray(inputs[k], dtype=np.float32))
        for k in ("Wn_w", "Wn_b", "We_w", "We_b", "Ws_w", "Ws_b")
    }
    h = np.asarray(inputs["h"], dtype=np.float32)
    adj = np.asarray(inputs["adj"], dtype=np.int32)
    ef = np.asarray(inputs["edge_feat"], dtype=np.float32)
    in_maps = []
    for c in range(NCORES):
        m = dict(w)
        m["h"] = np.ascontiguousarray(h[c])
        m["adj"] = np.ascontiguousarray(adj[c])
        m["edge_feat"] = np.ascontiguousarray(ef[c])
        in_maps.append(m)
    return in_maps


def run(inputs, trace=False):
    """Run on hardware; returns (full_output, BassKernelResults)."""
    nc = _get_nc()
    res = run_bass_kernel_spmd(nc, make_in_maps(inputs), list(range(NCORES)), trace=trace)
    out = np.stack(
        [np.asarray(res.results[c]["out"]) for c in range(NCORES)], axis=0
    ).astype(np.float32)
    return out, res


def kernel(**inputs):
    out, _ = run(inputs)
    return out


# revision 66
# speedup vs baseline: 1.4349x; 1.4349x over previous
"""Trainium2 Bass kernel for a GCN layer (gnn_message_passing).

Reference computation (per batch b):
    node_msg = h @ Wn_w.T + Wn_b                  # (N, OD)
    edge_msg = edge_feat @ We_w.T + We_b          # (N, N, OD)
    self_msg = h @ Ws_w.T + Ws_b                  # (N, OD)
    deg      = adj.sum(-1)                        # (N,)
    agg      = node_msg * deg + einsum('ij,ijo->io', adj, edge_msg)
    out      = relu(agg / clip(deg, 1) + self_msg)

Key algebraic rewrite: the (N,N,OD) edge_msg is never materialized.
    einsum('ij,ijo->io', adj, edge_feat @ We.T + We_b)
      = (einsum('ij,ije->ie', adj, edge_feat)) @ We.T + deg * We_b
so the dominant work is the adj-masked reduction of edge_feat over the
source-node axis j, producing (N, ED), followed by a tiny 16->64 matmul.

Sharding: data-parallel over batch B=8 across the 8 NeuronCores (one
batch element per core); weights replicated.

v2 pipeline design (per core) — three concurrent DMA queues:
  DMA transfers serialize on the ISSUING engine's queue (the engine is
  blocked for the whole transfer), but different queues' transfers
  overlap freely.  So the 16 MiB ef stream is split by j across the SP
  HWDGE queue (j < JS, ~58%) and the Pool SWDGE queue (j >= JS, ~42%),
  running concurrently (~2x the single-queue stream rate); ACT's queue
  carries the small tensors (adj / h / weights / biases), coalesced into
  multi-block DMAs, and the out DMAs ride the then-idle SP queue.

  Masked reduce  msum[i,e] = sum_j ef[i,j,e] * adj[i,j]  consumes the
  raw int32 adj directly (ALU converts on read — no widened copy, so
  nothing reduce-side waits on ACT), split by channel:
    e in [0, NA):   DVE scalar_tensor_tensor with accum_out, one instr
                    per (block, j-range).
    e in [NA, 16):  Pool tensor_tensor product (adj broadcast over the
                    channel axis via a stride-0 AP) into a prod tile,
                    then ACT activation(Copy, accum_out) per channel.
  Block 3 (the tail block) instead splits its channels DVE-stt / ACT /
  Pool pairwise-add TREE (9 halving levels over the prod tile): by the
  time its data lands (~30us) Pool's stream share is done, so the tree
  runs on an otherwise-idle engine while DVE drains its stt backlog.
  Degree rides ACT as activation(Copy, accum) on int32 adj, off the
  critical path.  Per-block glue: PE transpose(msum) -> copy -> (16->64)
  matmul -> (r*pes)+base -> relu, with base = degr*(h Wn^T + bn + be) +
  h Ws^T + bs precomputed mid-stream.  Block 0's Pool chunk and block
  3's SP chunk are split so the pipeline head starts early (~2.7us) and
  the serial tail after the final DMA stays short.

  CoreSim cost-model time: 40421 ns/core (baseline v1: 61159 ns).
  Engine busy: SP ~30us, Pool ~33us, ACT ~28us, DVE ~31us.
"""

import os
import sys
from contextlib import ExitStack

import numpy as np


def _ensure_concourse():
    try:
        import concourse  # noqa: F401
        return
    except ImportError:
        pass
    for p in ("/opt/trn_rl_repo", "/root/.axon_site/_ro/trn_rl_repo"):
        if os.path.isdir(p) and p not in sys.path:
            sys.path.insert(0, p)
            try:
                import concourse  # noqa: F401
                return
            except ImportError:
                continue
    raise ImportError("cannot locate the concourse (bass) package")


_ensure_concourse()

import concourse.bacc as bacc  # noqa: E402
import concourse.bass as bass  # noqa: E402
import concourse.tile as tile  # noqa: E402
from concourse import mybir  # noqa: E402
from concourse.bass_utils import run_bass_kernel_spmd  # noqa: E402
from concourse.masks import make_identity  # noqa: E402

B, N, ND, ED, OD = 8, 512, 64, 16, 64
NCORES = 8
PB = 128           # destination-node block (SBUF partitions)
NBLK = N // PB     # 4

F32 = mybir.dt.float32
I32 = mybir.dt.int32

# --- tuning knobs ---
JS = 295           # j < JS streams on the SP queue; j >= JS on Pool SWDGE
NA = 10            # channels 0..NA-1 via DVE stt; NA..15 via Pool+ACT
NC = ED - NA
NA3 = 7            # block 3 channels 0..NA3-1: DVE stt
NB3 = 2            # block 3 channels NA3..NA3+NB3-1: ACT activation-reduce
NC3 = ED - NA3     # block 3 channels NA3..15: Pool product (+tree for the
                   # last NC3-NB3 of them)
JS3A = 220         # block 3's SP chunk is split [0,JS3A) + [JS3A,JS) so the
                   # final arriving piece (and its serial compute) is small
JACT3 = 0          # first JACT3 j's of block 3 ride the ACT queue mid-stream
SPLIT_GLUE3 = False  # two-piece transpose/copy/matmul for block 3's glue
EF0_SPLIT = True   # split block 0's Pool chunk for an early DVE start

AF_COPY = mybir.ActivationFunctionType.Copy
MULT = mybir.AluOpType.mult
ADD = mybir.AluOpType.add
BYPASS = mybir.AluOpType.bypass


def _row_ap(handle, n):
    """View a 1-D DRAM tensor (n,) as a (1, n) AP."""
    ap = handle.ap()
    return bass.AP(tensor=ap.tensor, offset=ap.offset, ap=[[0, 1], [1, n]])


def _view(ap, axes, extra_offset=0):
    """Re-stride an AP (offset in elements)."""
    return bass.AP(tensor=ap.tensor, offset=ap.offset + extra_offset, ap=axes)


def build_bass(repeat=1, mode="full", unroll=1):
    """Build the single-core Bass program (SPMD across 8 cores)."""
    nc = bacc.Bacc(
        "TRN2",
        target_bir_lowering=False,
        debug=False,
        num_devices=NCORES,
    )

    h_d = nc.dram_tensor("h", [N, ND], F32, kind="ExternalInput")
    adj_d = nc.dram_tensor("adj", [N, N], I32, kind="ExternalInput")
    ef_d = nc.dram_tensor("edge_feat", [N, N, ED], F32, kind="ExternalInput")
    wn_d = nc.dram_tensor("Wn_w", [OD, ND], F32, kind="ExternalInput")
    wnb_d = nc.dram_tensor("Wn_b", [OD], F32, kind="ExternalInput")
    we_d = nc.dram_tensor("We_w", [OD, ED], F32, kind="ExternalInput")
    web_d = nc.dram_tensor("We_b", [OD], F32, kind="ExternalInput")
    ws_d = nc.dram_tensor("Ws_w", [OD, ND], F32, kind="ExternalInput")
    wsb_d = nc.dram_tensor("Ws_b", [OD], F32, kind="ExternalInput")
    out_d = nc.dram_tensor("out", [N, OD], F32, kind="ExternalOutput")

    h_ap = h_d.ap()
    adj_ap = adj_d.ap()
    ef_ap = ef_d.ap()
    out_ap = out_d.ap()

    with tile.TileContext(nc) as tc, ExitStack() as ctx:
        consts = ctx.enter_context(tc.tile_pool(name="consts", bufs=1))
        efp = ctx.enter_context(tc.tile_pool(name="efp", bufs=3))
        adjp = ctx.enter_context(tc.tile_pool(name="adjp", bufs=1))
        work = ctx.enter_context(tc.tile_pool(name="work", bufs=1))
        prodp = ctx.enter_context(tc.tile_pool(name="prodp", bufs=2))
        prod3p = ctx.enter_context(tc.tile_pool(name="prod3p", bufs=1))
        pset = ctx.enter_context(tc.tile_pool(name="pset", bufs=2, space="PSUM"))
        pmm = ctx.enter_context(tc.tile_pool(name="pmm", bufs=2, space="PSUM"))
        pep = ctx.enter_context(tc.tile_pool(name="pep", bufs=2, space="PSUM"))

        def emit_body():
            ident = consts.tile([128, 128], F32)
            make_identity(nc, ident)

            # ---- ACT queue head: adj first (its widen gates the whole
            # reduce pipeline), then weights/h/biases ----
            # adj: block 0's rows first (gates the first stt piece), then
            # blocks 1-3 coalesced (partition p holds rows {128b + p}).
            adj_t = adjp.tile([PB, NBLK, N], I32, tag="adjt")
            nc.scalar.dma_start(
                out=adj_t[:, 0, :],
                in_=_view(adj_ap, [[N, PB], [1, N]]),
            )
            nc.scalar.dma_start(
                out=adj_t[:, 1:NBLK, :],
                in_=_view(adj_ap, [[N, PB], [N * PB, NBLK - 1], [1, N]],
                          extra_offset=N * PB),
            )

            # weights + h + biases on ACT
            wn_sb = consts.tile([OD, ND], F32, tag="wload")
            nc.scalar.dma_start(out=wn_sb, in_=wn_d.ap())
            ws_sb = consts.tile([OD, ND], F32, tag="wload2")
            nc.scalar.dma_start(out=ws_sb, in_=ws_d.ap())
            we_sb = consts.tile([OD, ED], F32, tag="wload3")
            nc.scalar.dma_start(out=we_sb, in_=we_d.ap())
            h_sb = work.tile([PB, NBLK, ND], F32, tag="hload")
            nc.scalar.dma_start(
                out=h_sb,
                in_=_view(h_ap, [[ND, PB], [ND * PB, NBLK], [1, ND]]),
            )
            bias_n = consts.tile([1, OD], F32)
            nc.scalar.dma_start(out=bias_n, in_=_row_ap(wnb_d, OD))
            bias_e = consts.tile([1, OD], F32)
            nc.scalar.dma_start(out=bias_e, in_=_row_ap(web_d, OD))

            # degree per block: ACT activation(Copy, accum) reading the raw
            # int32 adj (the reduce channels consume int32 adj directly, so
            # no widened copy is needed and this pass is off the critical
            # path -- deg only feeds base/r, used mid-stream).
            deg_scr = work.tile([PB, N], F32, tag="degscr")
            degs = []
            for ib in range(NBLK):
                deg = work.tile([PB, 1], F32, tag=f"deg{ib}")
                nc.scalar.activation(
                    out=deg_scr,
                    in_=adj_t[:, ib, :],
                    func=AF_COPY,
                    accum_out=deg,
                )
                degs.append(deg)

            # ---- ef stream: SP gets j<JS per block; Pool SWDGE the rest.
            # SP's DMAs all go up front (its queue has nothing else); Pool's
            # queue interleaves its ef DMAs with the product work so the
            # products don't all queue behind the stream.  Block 3's SP
            # chunk is split so the final arriving piece is small.
            ef_ts = [
                efp.tile([PB, N, ED], F32, tag="ef", name=f"ef_t{ib}")
                for ib in range(NBLK)
            ]
            sp_pieces = {0: [(0, JS)], 1: [(0, JS)], 2: [(0, JS)],
                         3: [(JACT3, JS3A - JACT3), (JS3A, JS - JS3A)]}
            for ib in range(NBLK):
                i0 = ib * PB
                for (j0, jn) in sp_pieces[ib]:
                    nc.sync.dma_start(
                        out=ef_ts[ib][:, j0 : j0 + jn, :],
                        in_=ef_ap[i0 : i0 + PB, j0 : j0 + jn, :],
                    )

            def pool_ef(ib, j0=JS, jn=N - JS):
                i0 = ib * PB
                nc.gpsimd.dma_start(
                    out=ef_ts[ib][:, j0 : j0 + jn, :],
                    in_=ef_ap[i0 : i0 + PB, j0 : j0 + jn, :],
                )

            prods = {}   # ib -> prod tile [PB, nc_b, N]

            def pool_prod(ib, j0, jn):
                """Pool products for block ib's ACT channels over [j0, j0+jn)."""
                na_b = NA3 if ib == 3 else NA
                nc_b = ED - na_b
                if ib not in prods:
                    pool = prod3p if ib == 3 else prodp
                    prods[ib] = pool.tile(
                        [PB, nc_b, N], F32, tag="prod", name=f"prod{ib}"
                    )
                prod = prods[ib]
                ef_apv = ef_ts[ib][:]
                af = adj_t[:, ib, :]
                prod_apv = prod[:]
                in0 = _view(ef_apv, [ef_apv.ap[0], [1, nc_b], [ED, jn]],
                            extra_offset=j0 * ED + na_b)
                in1 = _view(af, [af.ap[0], [0, nc_b], [1, jn]], extra_offset=j0)
                outp_ = _view(prod_apv, [prod_apv.ap[0], [N, nc_b], [1, jn]],
                              extra_offset=j0)
                nc.gpsimd.tensor_tensor(out=outp_, in0=in0, in1=in1, op=MULT)

            # Block 3's ACT channels are reduced by a pairwise-add tree on
            # Pool (idle once its stream ends) instead of ACT activations.
            tree_a = work.tile([PB, NC3, N // 2], F32, tag="treea")
            tree_b = work.tile([PB, NC3, N // 4], F32, tag="treeb")

            def pool_tree(ib, cb=0, target=None, tcol=None):
                """Pairwise-add tree on Pool over prod channels [cb, nc_b)
                (idle once the Pool stream ends), writing msum columns."""
                prod = prods[ib]
                na_b = NA3 if ib == 3 else NA
                nc_b = ED - na_b
                if target is None:
                    msum, t0_ = msums[ib], na_b + cb
                else:
                    msum, t0_ = target, tcol
                nt = nc_b - cb                  # tree channel count
                lvl_eng = nc.gpsimd             # engine for all levels
                src = prod[:]
                w = N // 2
                lvl_eng.tensor_tensor(
                    out=_view(tree_a[:], [tree_a[:].ap[0], [w, nt], [1, w]]),
                    in0=_view(src, [src.ap[0], [src.ap[1][0], nt], [1, w]],
                              extra_offset=cb * src.ap[1][0]),
                    in1=_view(src, [src.ap[0], [src.ap[1][0], nt], [1, w]],
                              extra_offset=cb * src.ap[1][0] + w),
                    op=ADD,
                )
                src = _view(tree_a[:], [tree_a[:].ap[0], [w, nt], [1, w]])
                src_n = w
                bufs_ = [tree_b, tree_a]
                k = 0
                while src_n > 2:
                    w = src_n // 2
                    dst = bufs_[k % 2]
                    lvl_eng.tensor_tensor(
                        out=_view(dst[:], [dst[:].ap[0], [w, nt], [1, w]]),
                        in0=_view(src, [src.ap[0], [src.ap[1][0], nt], [1, w]]),
                        in1=_view(src, [src.ap[0], [src.ap[1][0], nt], [1, w]],
                                  extra_offset=w),
                        op=ADD,
                    )
                    src = _view(dst[:], [dst[:].ap[0], [w, nt], [1, w]])
                    src_n = w
                    k += 1
                # final 2 -> 1 straight into msum columns
                msv = msum[:]
                lvl_eng.tensor_tensor(
                    out=_view(msv, [msv.ap[0], [1, nt], [1, 1]],
                              extra_offset=t0_),
                    in0=_view(src, [src.ap[0], [src.ap[1][0], nt], [1, 1]]),
                    in1=_view(src, [src.ap[0], [src.ap[1][0], nt], [1, 1]],
                              extra_offset=1),
                    op=ADD,
                )

            # Pool queue order: two ef DMAs ahead, then alternate.  Block
            # 0's chunk is split so DVE's first stt piece can start ~2.7us.
            if EF0_SPLIT:
                pool_ef(0, JS, 64)
                pool_ef(0, JS + 64, N - JS - 64)
            else:
                pool_ef(0)
            pool_ef(1)
            pool_prod(0, 0, N)
            pool_ef(2)
            pool_prod(1, 0, N)
            pool_ef(3)
            pool_prod(2, 0, N)
            pool_prod(3, 0, JS3A)
            pool_prod(3, JS3A, N - JS3A)
            # pool_tree(3) is emitted in the interleaved section below (no
            # other Pool instructions are emitted in between, so it still
            # lands right after prod3 in Pool's queue).

            # ---- weight prep: transpose on PE; biases folded as extra row --
            rhs_n = consts.tile([ND + 1, OD], F32)
            rhs_s = consts.tile([ND + 1, OD], F32)
            weT = consts.tile([ED, OD], F32)

            pw = pset.tile([ND, OD], F32, tag="t")
            nc.tensor.transpose(pw, wn_sb, ident[:ND, :OD])
            nc.vector.tensor_copy(out=rhs_n[0:ND, :], in_=pw)
            pw2 = pset.tile([ND, OD], F32, tag="t")
            nc.tensor.transpose(pw2, ws_sb, ident[:ND, :OD])
            nc.vector.tensor_copy(out=rhs_s[0:ND, :], in_=pw2)
            pw3 = pset.tile([ED, OD], F32, tag="t")
            nc.tensor.transpose(pw3, we_sb, ident[:ND, :OD])
            nc.vector.tensor_copy(out=weT, in_=pw3)
            # weT rows [NA3+NB3, ED) re-based at partition 0 for block 3's
            # split projection
            weT_b = consts.tile([ED, OD], F32, tag="weTb")
            pw3b = pset.tile([ED, OD], F32, tag="t")
            nc.tensor.transpose(
                pw3b[0 : ED - NA3 - NB3, :],
                we_sb[:, NA3 + NB3 : ED],
                ident[:ND, :OD],
            )
            nc.vector.tensor_copy(
                out=weT_b[0 : ED - NA3 - NB3, :],
                in_=pw3b[0 : ED - NA3 - NB3, :],
            )

            nc.vector.tensor_add(rhs_n[ND : ND + 1, :], bias_n, bias_e)
            nc.scalar.dma_start(out=rhs_s[ND : ND + 1, :], in_=_row_ap(wsb_d, OD))

            hT = consts.tile([ND + 1, N], F32)
            nc.vector.memset(hT[ND : ND + 1, :], 1.0)
            rs, degrs, bases = [], [], []

            def emit_mid():
                """Deg chain, h^T and base precompute -- DVE work that
                depends on the ACT widens / h DMA, emitted after stt(0) so
                the DVE queue head never stalls on it."""
                # degc = max(deg,1); r = 1/degc; degr = deg*r
                for ib in range(NBLK):
                    degc = work.tile([PB, 1], F32, tag=f"degc{ib}")
                    nc.vector.tensor_scalar_max(degc, degs[ib], 1.0)
                    r = work.tile([PB, 1], F32, tag=f"r{ib}")
                    nc.vector.reciprocal(r, degc)
                    degr = work.tile([PB, 1], F32, tag=f"degr{ib}")
                    nc.vector.tensor_mul(degr, degs[ib], r)
                    rs.append(r)
                    degrs.append(degr)
                # h^T with an appended ones-row: (65, 512)
                for ib in range(NBLK):
                    ph = pset.tile([ND, PB], F32, tag="t")
                    nc.tensor.transpose(ph, h_sb[:, ib, :], ident)
                    nc.vector.tensor_copy(
                        out=hT[0:ND, ib * PB : (ib + 1) * PB], in_=ph
                    )
                # base = degr*(node+biases) + self
                for ib in range(NBLK):
                    i0 = ib * PB
                    pn = pmm.tile([PB, OD], F32, tag="pn")
                    nc.tensor.matmul(
                        pn, lhsT=hT[:, i0 : i0 + PB], rhs=rhs_n,
                        start=True, stop=True,
                    )
                    hs = pmm.tile([PB, OD], F32, tag="hs")
                    nc.tensor.matmul(
                        hs, lhsT=hT[:, i0 : i0 + PB], rhs=rhs_s,
                        start=True, stop=True,
                    )
                    an = work.tile([PB, OD], F32, tag=f"an{ib}")
                    nc.scalar.activation(
                        out=an, in_=pn, func=AF_COPY, scale=degrs[ib]
                    )
                    base = work.tile([PB, OD], F32, tag=f"base{ib}")
                    nc.vector.tensor_add(base, an, hs)
                    bases.append(base)

            # ---- per-block masked reduce + combine ----
            scr = work.tile([PB, N], F32, tag="scr")       # stt throwaway out
            scr_a = work.tile([PB, N], F32, tag="scra")    # ACT reduce out
            ob = work.tile([PB, NBLK, OD], F32, tag="ob")  # merged output
            msums = {
                ib: work.tile([PB, ED], F32, tag=f"msum{ib}", name=f"msum{ib}")
                for ib in range(NBLK)
            }
            msum3_t = work.tile([PB, ED], F32, tag="msum3t")

            def emit_stt(ib):
                """DVE stt channels -> msum[:, 0:na_b].  Whole-j per channel
                for blocks 0-2; block 3 in two pieces (tail)."""
                ef_t = ef_ts[ib]
                af = adj_t[:, ib, :]
                msum = msums[ib]
                na_b = NA3 if ib == 3 else NA
                if 0 < ib < 3:
                    for e in range(na_b):
                        nc.vector.scalar_tensor_tensor(
                            out=scr[:, 0:N],
                            in0=ef_t[:, :, e],
                            scalar=1.0,
                            in1=af,
                            op0=BYPASS,
                            op1=MULT,
                            accum_out=msum[:, e : e + 1],
                        )
                else:
                    # blocks 0 and 3 reduce in pieces: block 0's Pool chunk
                    # arrives first (split again so DVE starts ~2.7us);
                    # block 3's split shortens the tail.
                    if ib == 0 and EF0_SPLIT:
                        pieces = ((JS, 64), (JS + 64, N - JS - 64), (0, JS))
                    elif ib == 0:
                        pieces = ((JS, N - JS), (0, JS))
                    elif JACT3 > 0:
                        pieces = ((0, JACT3), (JACT3, JS3A - JACT3),
                                  (JS3A, N - JS3A))
                    else:
                        pieces = ((0, JS3A), (JS3A, N - JS3A))
                    parts = []
                    for (j0, jn) in pieces:
                        pA = work.tile([PB, na_b], F32, tag=f"pA{ib}_{j0}")
                        for e in range(na_b):
                            nc.vector.scalar_tensor_tensor(
                                out=scr[:, 0:jn],
                                in0=ef_t[:, j0 : j0 + jn, e],
                                scalar=1.0,
                                in1=af[:, j0 : j0 + jn],
                                op0=BYPASS,
                                op1=MULT,
                                accum_out=pA[:, e : e + 1],
                            )
                        parts.append(pA)
                    acc = parts[0]
                    for k in range(1, len(parts) - 1):
                        nxt = work.tile([PB, na_b], F32, tag=f"pacc{ib}_{k}",
                                        name=f"pacc{ib}_{k}")
                        nc.vector.tensor_add(nxt, acc, parts[k])
                        acc = nxt
                    nc.vector.tensor_add(msum[:, 0:na_b], acc, parts[-1])

            def emit_red(ib):
                """ACT activation reduce of Pool products -> msum[:, na_b:]."""
                msum = msums[ib]
                na_b = NA3 if ib == 3 else NA
                nc_b = ED - na_b
                prod = prods[ib]
                if ib < 3:
                    for c in range(nc_b):
                        nc.scalar.activation(
                            out=scr_a[:, 0:N],
                            in_=prod[:, c, :],
                            func=AF_COPY,
                            accum_out=msum[:, na_b + c : na_b + c + 1],
                        )
                else:
                    # block 3: NB3 channels on ACT (full-range, one instr
                    # each), the rest via the Pool pairwise-add tree
                    for c in range(NB3):
                        nc.scalar.activation(
                            out=scr_a[:, 0:N],
                            in_=prod[:, c, :],
                            func=AF_COPY,
                            accum_out=msum[:, na_b + c : na_b + c + 1],
                        )
                    if SPLIT_GLUE3:
                        pool_tree(ib, cb=NB3, target=msum3_t, tcol=0)
                    else:
                        pool_tree(ib, cb=NB3)

            def emit_glue(ib):
                """(128,16) -> (16,128), project with We^T, combine, relu."""
                msum = msums[ib]
                pm = pset.tile([ED, PB], F32, tag="t")
                msT = work.tile([ED, PB], F32, tag=f"msT{ib}")
                pes = pep.tile([PB, OD], F32, tag="pes")
                if ib == 3 and SPLIT_GLUE3:
                    # two-piece transpose+copy+matmul: the stt/ACT columns
                    # are done well before the tree's -- only the tree's nt
                    # columns remain on the critical tail.  The projection
                    # accumulates two partial matmuls in PSUM.
                    cb2 = NA3 + NB3
                    nt = ED - cb2
                    nc.tensor.transpose(pm[0:cb2, :], msum[:, 0:cb2], ident)
                    nc.scalar.copy(out=msT[0:cb2, :], in_=pm[0:cb2, :])
                    nc.tensor.matmul(pes, lhsT=msT[0:cb2, :], rhs=weT[0:cb2, :],
                                     start=True, stop=False)
                    pm2 = pset.tile([ED, PB], F32, tag="t")
                    msT_b = work.tile([ED, PB], F32, tag="msTb")
                    nc.tensor.transpose(pm2[0:nt, :], msum3_t[:, 0:nt], ident)
                    nc.scalar.copy(out=msT_b[0:nt, :], in_=pm2[0:nt, :])
                    nc.tensor.matmul(pes, lhsT=msT_b[0:nt, :], rhs=weT_b[0:nt, :],
                                     start=False, stop=True)
                else:
                    nc.tensor.transpose(pm, msum, ident)
                    nc.scalar.copy(out=msT, in_=pm)
                    nc.tensor.matmul(pes, lhsT=msT, rhs=weT, start=True, stop=True)

                nc.vector.scalar_tensor_tensor(
                    out=ob[:, ib, :],
                    in0=pes,
                    scalar=rs[ib],
                    in1=bases[ib],
                    op0=MULT,
                    op1=ADD,
                )
                nc.vector.tensor_scalar_max(ob[:, ib, :], ob[:, ib, :], 0.0)

            # software-pipelined emission: stt b+1 ahead of glue b so DVE
            # never stalls on a glue chain waiting for ACT.
            emit_stt(0)
            emit_mid()
            emit_red(0)
            emit_stt(1)
            emit_glue(0)
            emit_red(1)
            if JACT3 > 0:
                # an early slice of block 3 rides ACT's mid-stream slack
                nc.scalar.dma_start(
                    out=ef_ts[3][:, 0:JACT3, :],
                    in_=ef_ap[3 * PB : 4 * PB, 0:JACT3, :],
                )
            emit_stt(2)
            emit_glue(1)
            emit_red(2)
            emit_stt(3)
            emit_glue(2)
            emit_red(3)
            emit_glue(3)

            # ---- out DMAs on the idle SP queue: blocks 0-2 as soon as
            # their relu lands, block 3's 32KB alone on the tail ----
            nc.sync.dma_start(
                out=_view(out_ap, [[OD, PB], [OD * PB, NBLK - 1], [1, OD]]),
                in_=ob[:, 0 : NBLK - 1, :],
            )
            nc.sync.dma_start(
                out=_view(out_ap, [[OD, PB], [1, OD]],
                          extra_offset=OD * PB * (NBLK - 1)),
                in_=ob[:, NBLK - 1, :],
            )

        if repeat == 1:
            for _ in range(unroll):
                emit_body()
        else:
            with tc.For_i(0, repeat, 1):
                for _ in range(unroll):
                    emit_body()

    nc.compile()
    return nc


_NC_CACHE = None


def _get_nc():
    global _NC_CACHE
    if _NC_CACHE is None:
        _NC_CACHE = build_bass()
    return _NC_CACHE


def make_in_maps(inputs):
    w = {
        k: np.ascontiguousarray(np.asarray(inputs[k], dtype=np.float32))
        for k in ("Wn_w", "Wn_b", "We_w", "We_b", "Ws_w", "Ws_b")
    }
    h = np.asarray(inputs["h"], dtype=np.float32)
    adj = np.asarray(inputs["adj"], dtype=np.int32)
    ef = np.asarray(inputs["edge_feat"], dtype=np.float32)
    in_maps = []
    for c in range(NCORES):
        m = dict(w)
        m["h"] = np.ascontiguousarray(h[c])
        m["adj"] = np.ascontiguousarray(adj[c])
        m["edge_feat"] = np.ascontiguousarray(ef[c])
        in_maps.append(m)
    return in_maps


def run(inputs, trace=False):
    """Run on hardware; returns (full_output, BassKernelResults)."""
    nc = _get_nc()
    res = run_bass_kernel_spmd(nc, make_in_maps(inputs), list(range(NCORES)), trace=trace)
    out = np.stack(
        [np.asarray(res.results[c]["out"]) for c in range(NCORES)], axis=0
    ).astype(np.float32)
    return out, res


def kernel(**inputs):
    out, _ = run(inputs)
    return out


# revision 71
# speedup vs baseline: 1.4598x; 1.0173x over previous
"""Trainium2 Bass kernel for a GCN layer (gnn_message_passing).

Reference computation (per batch b):
    node_msg = h @ Wn_w.T + Wn_b                  # (N, OD)
    edge_msg = edge_feat @ We_w.T + We_b          # (N, N, OD)
    self_msg = h @ Ws_w.T + Ws_b                  # (N, OD)
    deg      = adj.sum(-1)                        # (N,)
    agg      = node_msg * deg + einsum('ij,ijo->io', adj, edge_msg)
    out      = relu(agg / clip(deg, 1) + self_msg)

Key algebraic rewrite: the (N,N,OD) edge_msg is never materialized.
    einsum('ij,ijo->io', adj, edge_feat @ We.T + We_b)
      = (einsum('ij,ije->ie', adj, edge_feat)) @ We.T + deg * We_b
so the dominant work is the adj-masked reduction of edge_feat over the
source-node axis j, producing (N, ED), followed by a tiny 16->64 matmul.

Sharding: data-parallel over batch B=8 across the 8 NeuronCores (one
batch element per core); weights replicated.

v2 pipeline design (per core) — three concurrent DMA queues:
  DMA transfers serialize on the ISSUING engine's queue (the engine is
  blocked for the whole transfer), but different queues' transfers
  overlap freely.  So the 16 MiB ef stream is split by j across the SP
  HWDGE queue (j < JS, ~58%) and the Pool SWDGE queue (j >= JS, ~42%),
  running concurrently (~2x the single-queue stream rate); ACT's queue
  carries the small tensors (adj / h / weights / biases), coalesced into
  multi-block DMAs, and the out DMAs ride the then-idle SP queue.

  Masked reduce  msum[i,e] = sum_j ef[i,j,e] * adj[i,j]  consumes the
  raw int32 adj directly (ALU converts on read — no widened copy, so
  nothing reduce-side waits on ACT), split by channel:
    e in [0, NA):   DVE scalar_tensor_tensor with accum_out, one instr
                    per (block, j-range).
    e in [NA, 16):  Pool tensor_tensor product (adj broadcast over the
                    channel axis via a stride-0 AP) into a prod tile,
                    then ACT activation(Copy, accum_out) per channel.
  Block 3 (the tail block) instead splits its channels DVE-stt / ACT /
  Pool pairwise-add TREE (9 halving levels over the prod tile): by the
  time its data lands (~30us) Pool's stream share is done, so the tree
  runs on an otherwise-idle engine while DVE drains its stt backlog.
  Degree rides ACT as activation(Copy, accum) on int32 adj, off the
  critical path.  Per-block glue: PE transpose(msum) -> copy -> (16->64)
  matmul -> (r*pes)+base -> relu, with base = degr*(h Wn^T + bn + be) +
  h Ws^T + bs precomputed mid-stream.  Block 0's Pool chunk and block
  3's SP chunk are split so the pipeline head starts early (~2.7us) and
  the serial tail after the final DMA stays short.

  ACT also carries a KA-wide slice of blocks 1/2's chunk in its
  mid-stream slack, relieving the Pool queue (the busiest engine).

  CoreSim cost-model time: 39732 ns/core (baseline v1: 61159 ns).
  Engine busy: SP ~30us, Pool ~32us, ACT ~30us, DVE ~31us.
"""

import os
import sys
from contextlib import ExitStack

import numpy as np


def _ensure_concourse():
    try:
        import concourse  # noqa: F401
        return
    except ImportError:
        pass
    for p in ("/opt/trn_rl_repo", "/root/.axon_site/_ro/trn_rl_repo"):
        if os.path.isdir(p) and p not in sys.path:
            sys.path.insert(0, p)
            try:
                import concourse  # noqa: F401
                return
            except ImportError:
                continue
    raise ImportError("cannot locate the concourse (bass) package")


_ensure_concourse()

import concourse.bacc as bacc  # noqa: E402
import concourse.bass as bass  # noqa: E402
import concourse.tile as tile  # noqa: E402
from concourse import mybir  # noqa: E402
from concourse.bass_utils import run_bass_kernel_spmd  # noqa: E402
from concourse.masks import make_identity  # noqa: E402

B, N, ND, ED, OD = 8, 512, 64, 16, 64
NCORES = 8
PB = 128           # destination-node block (SBUF partitions)
NBLK = N // PB     # 4

F32 = mybir.dt.float32
I32 = mybir.dt.int32

# --- tuning knobs ---
JS = 295           # j < JS streams on the SP queue; j >= JS on Pool SWDGE
NA = 10            # channels 0..NA-1 via DVE stt; NA..15 via Pool+ACT
NC = ED - NA
NA3 = 7            # block 3 channels 0..NA3-1: DVE stt
NB3 = 2            # block 3 channels NA3..NA3+NB3-1: ACT activation-reduce
NC3 = ED - NA3     # block 3 channels NA3..15: Pool product (+tree for the
                   # last NC3-NB3 of them)
JS3A = 220         # block 3's SP chunk is split [0,JS3A) + [JS3A,JS) so the
                   # final arriving piece (and its serial compute) is small
JACT3 = 0          # first JACT3 j's of block 3 ride the ACT queue mid-stream
SPLIT_GLUE3 = False  # two-piece transpose/copy/matmul for block 3's glue
EF0_SPLIT = True   # split block 0's Pool chunk for an early DVE start
EF0A = 64          # size of block 0's first Pool piece
JS3 = 295          # block 3's SP/Pool j boundary (< JS shifts tail bytes to
                   # Pool, whose b3 chunk lands early)
KA = 32            # j's of blocks 1/2 carried by ACT's mid-stream slack

AF_COPY = mybir.ActivationFunctionType.Copy
MULT = mybir.AluOpType.mult
ADD = mybir.AluOpType.add
BYPASS = mybir.AluOpType.bypass


def _row_ap(handle, n):
    """View a 1-D DRAM tensor (n,) as a (1, n) AP."""
    ap = handle.ap()
    return bass.AP(tensor=ap.tensor, offset=ap.offset, ap=[[0, 1], [1, n]])


def _view(ap, axes, extra_offset=0):
    """Re-stride an AP (offset in elements)."""
    return bass.AP(tensor=ap.tensor, offset=ap.offset + extra_offset, ap=axes)


def build_bass(repeat=1, mode="full", unroll=1):
    """Build the single-core Bass program (SPMD across 8 cores)."""
    nc = bacc.Bacc(
        "TRN2",
        target_bir_lowering=False,
        debug=False,
        num_devices=NCORES,
    )

    h_d = nc.dram_tensor("h", [N, ND], F32, kind="ExternalInput")
    adj_d = nc.dram_tensor("adj", [N, N], I32, kind="ExternalInput")
    ef_d = nc.dram_tensor("edge_feat", [N, N, ED], F32, kind="ExternalInput")
    wn_d = nc.dram_tensor("Wn_w", [OD, ND], F32, kind="ExternalInput")
    wnb_d = nc.dram_tensor("Wn_b", [OD], F32, kind="ExternalInput")
    we_d = nc.dram_tensor("We_w", [OD, ED], F32, kind="ExternalInput")
    web_d = nc.dram_tensor("We_b", [OD], F32, kind="ExternalInput")
    ws_d = nc.dram_tensor("Ws_w", [OD, ND], F32, kind="ExternalInput")
    wsb_d = nc.dram_tensor("Ws_b", [OD], F32, kind="ExternalInput")
    out_d = nc.dram_tensor("out", [N, OD], F32, kind="ExternalOutput")

    h_ap = h_d.ap()
    adj_ap = adj_d.ap()
    ef_ap = ef_d.ap()
    out_ap = out_d.ap()

    with tile.TileContext(nc) as tc, ExitStack() as ctx:
        consts = ctx.enter_context(tc.tile_pool(name="consts", bufs=1))
        efp = ctx.enter_context(tc.tile_pool(name="efp", bufs=3))
        adjp = ctx.enter_context(tc.tile_pool(name="adjp", bufs=1))
        work = ctx.enter_context(tc.tile_pool(name="work", bufs=1))
        prodp = ctx.enter_context(tc.tile_pool(name="prodp", bufs=2))
        prod3p = ctx.enter_context(tc.tile_pool(name="prod3p", bufs=1))
        pset = ctx.enter_context(tc.tile_pool(name="pset", bufs=2, space="PSUM"))
        pmm = ctx.enter_context(tc.tile_pool(name="pmm", bufs=2, space="PSUM"))
        pep = ctx.enter_context(tc.tile_pool(name="pep", bufs=2, space="PSUM"))

        def emit_body():
            ident = consts.tile([128, 128], F32)
            make_identity(nc, ident)

            # ---- ACT queue head: adj first (its widen gates the whole
            # reduce pipeline), then weights/h/biases ----
            # adj: block 0's rows first (gates the first stt piece), then
            # blocks 1-3 coalesced (partition p holds rows {128b + p}).
            adj_t = adjp.tile([PB, NBLK, N], I32, tag="adjt")
            nc.scalar.dma_start(
                out=adj_t[:, 0, :],
                in_=_view(adj_ap, [[N, PB], [1, N]]),
            )
            nc.scalar.dma_start(
                out=adj_t[:, 1:NBLK, :],
                in_=_view(adj_ap, [[N, PB], [N * PB, NBLK - 1], [1, N]],
                          extra_offset=N * PB),
            )

            # weights + h + biases on ACT
            wn_sb = consts.tile([OD, ND], F32, tag="wload")
            nc.scalar.dma_start(out=wn_sb, in_=wn_d.ap())
            ws_sb = consts.tile([OD, ND], F32, tag="wload2")
            nc.scalar.dma_start(out=ws_sb, in_=ws_d.ap())
            we_sb = consts.tile([OD, ED], F32, tag="wload3")
            nc.scalar.dma_start(out=we_sb, in_=we_d.ap())
            h_sb = work.tile([PB, NBLK, ND], F32, tag="hload")
            nc.scalar.dma_start(
                out=h_sb,
                in_=_view(h_ap, [[ND, PB], [ND * PB, NBLK], [1, ND]]),
            )
            bias_n = consts.tile([1, OD], F32)
            nc.scalar.dma_start(out=bias_n, in_=_row_ap(wnb_d, OD))
            bias_e = consts.tile([1, OD], F32)
            nc.scalar.dma_start(out=bias_e, in_=_row_ap(web_d, OD))

            # degree per block: ACT activation(Copy, accum) reading the raw
            # int32 adj (the reduce channels consume int32 adj directly, so
            # no widened copy is needed and this pass is off the critical
            # path -- deg only feeds base/r, used mid-stream).
            deg_scr = work.tile([PB, N], F32, tag="degscr")
            degs = []
            for ib in range(NBLK):
                deg = work.tile([PB, 1], F32, tag=f"deg{ib}")
                nc.scalar.activation(
                    out=deg_scr,
                    in_=adj_t[:, ib, :],
                    func=AF_COPY,
                    accum_out=deg,
                )
                degs.append(deg)

            # ---- ef stream: SP gets j<JS per block; Pool SWDGE the rest.
            # SP's DMAs all go up front (its queue has nothing else); Pool's
            # queue interleaves its ef DMAs with the product work so the
            # products don't all queue behind the stream.  Block 3's SP
            # chunk is split so the final arriving piece is small.
            ef_ts = [
                efp.tile([PB, N, ED], F32, tag="ef", name=f"ef_t{ib}")
                for ib in range(NBLK)
            ]
            sp_pieces = {0: [(0, JS)], 1: [(0, JS)], 2: [(0, JS)],
                         3: [(JACT3, JS3A - JACT3), (JS3A, JS3 - JS3A)]}
            for ib in range(NBLK):
                i0 = ib * PB
                for (j0, jn) in sp_pieces[ib]:
                    nc.sync.dma_start(
                        out=ef_ts[ib][:, j0 : j0 + jn, :],
                        in_=ef_ap[i0 : i0 + PB, j0 : j0 + jn, :],
                    )

            def pool_ef(ib, j0=JS, jn=N - JS):
                i0 = ib * PB
                nc.gpsimd.dma_start(
                    out=ef_ts[ib][:, j0 : j0 + jn, :],
                    in_=ef_ap[i0 : i0 + PB, j0 : j0 + jn, :],
                )

            prods = {}   # ib -> prod tile [PB, nc_b, N]

            def pool_prod(ib, j0, jn):
                """Pool products for block ib's ACT channels over [j0, j0+jn)."""
                na_b = NA3 if ib == 3 else NA
                nc_b = ED - na_b
                if ib not in prods:
                    pool = prod3p if ib == 3 else prodp
                    prods[ib] = pool.tile(
                        [PB, nc_b, N], F32, tag="prod", name=f"prod{ib}"
                    )
                prod = prods[ib]
                ef_apv = ef_ts[ib][:]
                af = adj_t[:, ib, :]
                prod_apv = prod[:]
                in0 = _view(ef_apv, [ef_apv.ap[0], [1, nc_b], [ED, jn]],
                            extra_offset=j0 * ED + na_b)
                in1 = _view(af, [af.ap[0], [0, nc_b], [1, jn]], extra_offset=j0)
                outp_ = _view(prod_apv, [prod_apv.ap[0], [N, nc_b], [1, jn]],
                              extra_offset=j0)
                nc.gpsimd.tensor_tensor(out=outp_, in0=in0, in1=in1, op=MULT)

            # Block 3's ACT channels are reduced by a pairwise-add tree on
            # Pool (idle once its stream ends) instead of ACT activations.
            tree_a = work.tile([PB, NC3, N // 2], F32, tag="treea")
            tree_b = work.tile([PB, NC3, N // 4], F32, tag="treeb")

            def pool_tree(ib, cb=0, target=None, tcol=None):
                """Pairwise-add tree on Pool over prod channels [cb, nc_b)
                (idle once the Pool stream ends), writing msum columns."""
                prod = prods[ib]
                na_b = NA3 if ib == 3 else NA
                nc_b = ED - na_b
                if target is None:
                    msum, t0_ = msums[ib], na_b + cb
                else:
                    msum, t0_ = target, tcol
                nt = nc_b - cb                  # tree channel count
                lvl_eng = nc.gpsimd             # engine for all levels
                src = prod[:]
                w = N // 2
                lvl_eng.tensor_tensor(
                    out=_view(tree_a[:], [tree_a[:].ap[0], [w, nt], [1, w]]),
                    in0=_view(src, [src.ap[0], [src.ap[1][0], nt], [1, w]],
                              extra_offset=cb * src.ap[1][0]),
                    in1=_view(src, [src.ap[0], [src.ap[1][0], nt], [1, w]],
                              extra_offset=cb * src.ap[1][0] + w),
                    op=ADD,
                )
                src = _view(tree_a[:], [tree_a[:].ap[0], [w, nt], [1, w]])
                src_n = w
                bufs_ = [tree_b, tree_a]
                k = 0
                while src_n > 2:
                    w = src_n // 2
                    dst = bufs_[k % 2]
                    lvl_eng.tensor_tensor(
                        out=_view(dst[:], [dst[:].ap[0], [w, nt], [1, w]]),
                        in0=_view(src, [src.ap[0], [src.ap[1][0], nt], [1, w]]),
                        in1=_view(src, [src.ap[0], [src.ap[1][0], nt], [1, w]],
                                  extra_offset=w),
                        op=ADD,
                    )
                    src = _view(dst[:], [dst[:].ap[0], [w, nt], [1, w]])
                    src_n = w
                    k += 1
                # final 2 -> 1 straight into msum columns
                msv = msum[:]
                lvl_eng.tensor_tensor(
                    out=_view(msv, [msv.ap[0], [1, nt], [1, 1]],
                              extra_offset=t0_),
                    in0=_view(src, [src.ap[0], [src.ap[1][0], nt], [1, 1]]),
                    in1=_view(src, [src.ap[0], [src.ap[1][0], nt], [1, 1]],
                              extra_offset=1),
                    op=ADD,
                )

            # Pool queue order: two ef DMAs ahead, then alternate.  Block
            # 0's chunk is split so DVE's first stt piece can start ~2.7us.
            if EF0_SPLIT:
                pool_ef(0, JS, EF0A)
                pool_ef(0, JS + EF0A, N - JS - EF0A)
            else:
                pool_ef(0)
            pool_ef(1, JS + KA, N - JS - KA)
            if KA > 0:
                nc.scalar.dma_start(
                    out=ef_ts[1][:, JS : JS + KA, :],
                    in_=ef_ap[1 * PB : 2 * PB, JS : JS + KA, :],
                )
            pool_prod(0, 0, N)
            pool_ef(2, JS + KA, N - JS - KA)
            if KA > 0:
                nc.scalar.dma_start(
                    out=ef_ts[2][:, JS : JS + KA, :],
                    in_=ef_ap[2 * PB : 3 * PB, JS : JS + KA, :],
                )
            pool_prod(1, 0, N)
            pool_ef(3, JS3, N - JS3)
            pool_prod(2, 0, N)
            pool_prod(3, 0, JS3A)
            pool_prod(3, JS3A, N - JS3A)
            # pool_tree(3) is emitted in the interleaved section below (no
            # other Pool instructions are emitted in between, so it still
            # lands right after prod3 in Pool's queue).

            # ---- weight prep: transpose on PE; biases folded as extra row --
            rhs_n = consts.tile([ND + 1, OD], F32)
            rhs_s = consts.tile([ND + 1, OD], F32)
            weT = consts.tile([ED, OD], F32)

            pw = pset.tile([ND, OD], F32, tag="t")
            nc.tensor.transpose(pw, wn_sb, ident[:ND, :OD])
            nc.vector.tensor_copy(out=rhs_n[0:ND, :], in_=pw)
            pw2 = pset.tile([ND, OD], F32, tag="t")
            nc.tensor.transpose(pw2, ws_sb, ident[:ND, :OD])
            nc.vector.tensor_copy(out=rhs_s[0:ND, :], in_=pw2)
            pw3 = pset.tile([ED, OD], F32, tag="t")
            nc.tensor.transpose(pw3, we_sb, ident[:ND, :OD])
            nc.vector.tensor_copy(out=weT, in_=pw3)
            # weT rows [NA3+NB3, ED) re-based at partition 0 for block 3's
            # split projection
            weT_b = consts.tile([ED, OD], F32, tag="weTb")
            pw3b = pset.tile([ED, OD], F32, tag="t")
            nc.tensor.transpose(
                pw3b[0 : ED - NA3 - NB3, :],
                we_sb[:, NA3 + NB3 : ED],
                ident[:ND, :OD],
            )
            nc.vector.tensor_copy(
                out=weT_b[0 : ED - NA3 - NB3, :],
                in_=pw3b[0 : ED - NA3 - NB3, :],
            )

            nc.vector.tensor_add(rhs_n[ND : ND + 1, :], bias_n, bias_e)
            nc.scalar.dma_start(out=rhs_s[ND : ND + 1, :], in_=_row_ap(wsb_d, OD))

            hT = consts.tile([ND + 1, N], F32)
            nc.vector.memset(hT[ND : ND + 1, :], 1.0)
            rs, degrs, bases = [], [], []

            def emit_mid():
                """Deg chain, h^T and base precompute -- DVE work that
                depends on the ACT widens / h DMA, emitted after stt(0) so
                the DVE queue head never stalls on it."""
                # degc = max(deg,1); r = 1/degc; degr = deg*r
                for ib in range(NBLK):
                    degc = work.tile([PB, 1], F32, tag=f"degc{ib}")
                    nc.vector.tensor_scalar_max(degc, degs[ib], 1.0)
                    r = work.tile([PB, 1], F32, tag=f"r{ib}")
                    nc.vector.reciprocal(r, degc)
                    degr = work.tile([PB, 1], F32, tag=f"degr{ib}")
                    nc.vector.tensor_mul(degr, degs[ib], r)
                    rs.append(r)
                    degrs.append(degr)
                # h^T with an appended ones-row: (65, 512)
                for ib in range(NBLK):
                    ph = pset.tile([ND, PB], F32, tag="t")
                    nc.tensor.transpose(ph, h_sb[:, ib, :], ident)
                    nc.vector.tensor_copy(
                        out=hT[0:ND, ib * PB : (ib + 1) * PB], in_=ph
                    )
                # base = degr*(node+biases) + self
                for ib in range(NBLK):
                    i0 = ib * PB
                    pn = pmm.tile([PB, OD], F32, tag="pn")
                    nc.tensor.matmul(
                        pn, lhsT=hT[:, i0 : i0 + PB], rhs=rhs_n,
                        start=True, stop=True,
                    )
                    hs = pmm.tile([PB, OD], F32, tag="hs")
                    nc.tensor.matmul(
                        hs, lhsT=hT[:, i0 : i0 + PB], rhs=rhs_s,
                        start=True, stop=True,
                    )
                    an = work.tile([PB, OD], F32, tag=f"an{ib}")
                    nc.scalar.activation(
                        out=an, in_=pn, func=AF_COPY, scale=degrs[ib]
                    )
                    base = work.tile([PB, OD], F32, tag=f"base{ib}")
                    nc.vector.tensor_add(base, an, hs)
                    bases.append(base)

            # ---- per-block masked reduce + combine ----
            scr = work.tile([PB, N], F32, tag="scr")       # stt throwaway out
            scr_a = work.tile([PB, N], F32, tag="scra")    # ACT reduce out
            ob = work.tile([PB, NBLK, OD], F32, tag="ob")  # merged output
            msums = {
                ib: work.tile([PB, ED], F32, tag=f"msum{ib}", name=f"msum{ib}")
                for ib in range(NBLK)
            }
            msum3_t = work.tile([PB, ED], F32, tag="msum3t")

            def emit_stt(ib):
                """DVE stt channels -> msum[:, 0:na_b].  Whole-j per channel
                for blocks 0-2; block 3 in two pieces (tail)."""
                ef_t = ef_ts[ib]
                af = adj_t[:, ib, :]
                msum = msums[ib]
                na_b = NA3 if ib == 3 else NA
                if 0 < ib < 3:
                    for e in range(na_b):
                        nc.vector.scalar_tensor_tensor(
                            out=scr[:, 0:N],
                            in0=ef_t[:, :, e],
                            scalar=1.0,
                            in1=af,
                            op0=BYPASS,
                            op1=MULT,
                            accum_out=msum[:, e : e + 1],
                        )
                else:
                    # blocks 0 and 3 reduce in pieces: block 0's Pool chunk
                    # arrives first (split again so DVE starts ~2.7us);
                    # block 3's split shortens the tail.
                    if ib == 0 and EF0_SPLIT:
                        pieces = ((JS, EF0A), (JS + EF0A, N - JS - EF0A),
                                  (0, JS))
                    elif ib == 0:
                        pieces = ((JS, N - JS), (0, JS))
                    elif JACT3 > 0:
                        pieces = ((0, JACT3), (JACT3, JS3A - JACT3),
                                  (JS3A, N - JS3A))
                    else:
                        pieces = ((0, JS3A), (JS3A, N - JS3A))
                    parts = []
                    for (j0, jn) in pieces:
                        pA = work.tile([PB, na_b], F32, tag=f"pA{ib}_{j0}")
                        for e in range(na_b):
                            nc.vector.scalar_tensor_tensor(
                                out=scr[:, 0:jn],
                                in0=ef_t[:, j0 : j0 + jn, e],
                                scalar=1.0,
                                in1=af[:, j0 : j0 + jn],
                                op0=BYPASS,
                                op1=MULT,
                                accum_out=pA[:, e : e + 1],
                            )
                        parts.append(pA)
                    acc = parts[0]
                    for k in range(1, len(parts) - 1):
                        nxt = work.tile([PB, na_b], F32, tag=f"pacc{ib}_{k}",
                                        name=f"pacc{ib}_{k}")
                        nc.vector.tensor_add(nxt, acc, parts[k])
                        acc = nxt
                    nc.vector.tensor_add(msum[:, 0:na_b], acc, parts[-1])

            def emit_red(ib):
                """ACT activation reduce of Pool products -> msum[:, na_b:]."""
                msum = msums[ib]
                na_b = NA3 if ib == 3 else NA
                nc_b = ED - na_b
                prod = prods[ib]
                if ib < 3:
                    for c in range(nc_b):
                        nc.scalar.activation(
                            out=scr_a[:, 0:N],
                            in_=prod[:, c, :],
                            func=AF_COPY,
                            accum_out=msum[:, na_b + c : na_b + c + 1],
                        )
                else:
                    # block 3: NB3 channels on ACT (full-range, one instr
                    # each), the rest via the Pool pairwise-add tree
                    for c in range(NB3):
                        nc.scalar.activation(
                            out=scr_a[:, 0:N],
                            in_=prod[:, c, :],
                            func=AF_COPY,
                            accum_out=msum[:, na_b + c : na_b + c + 1],
                        )
                    if SPLIT_GLUE3:
                        pool_tree(ib, cb=NB3, target=msum3_t, tcol=0)
                    else:
                        pool_tree(ib, cb=NB3)

            def emit_glue(ib):
                """(128,16) -> (16,128), project with We^T, combine, relu."""
                msum = msums[ib]
                pm = pset.tile([ED, PB], F32, tag="t")
                msT = work.tile([ED, PB], F32, tag=f"msT{ib}")
                pes = pep.tile([PB, OD], F32, tag="pes")
                if ib == 3 and SPLIT_GLUE3:
                    # two-piece transpose+copy+matmul: the stt/ACT columns
                    # are done well before the tree's -- only the tree's nt
                    # columns remain on the critical tail.  The projection
                    # accumulates two partial matmuls in PSUM.
                    cb2 = NA3 + NB3
                    nt = ED - cb2
                    nc.tensor.transpose(pm[0:cb2, :], msum[:, 0:cb2], ident)
                    nc.scalar.copy(out=msT[0:cb2, :], in_=pm[0:cb2, :])
                    nc.tensor.matmul(pes, lhsT=msT[0:cb2, :], rhs=weT[0:cb2, :],
                                     start=True, stop=False)
                    pm2 = pset.tile([ED, PB], F32, tag="t")
                    msT_b = work.tile([ED, PB], F32, tag="msTb")
                    nc.tensor.transpose(pm2[0:nt, :], msum3_t[:, 0:nt], ident)
                    nc.scalar.copy(out=msT_b[0:nt, :], in_=pm2[0:nt, :])
                    nc.tensor.matmul(pes, lhsT=msT_b[0:nt, :], rhs=weT_b[0:nt, :],
                                     start=False, stop=True)
                else:
                    nc.tensor.transpose(pm, msum, ident)
                    nc.scalar.copy(out=msT, in_=pm)
                    nc.tensor.matmul(pes, lhsT=msT, rhs=weT, start=True, stop=True)

                nc.vector.scalar_tensor_tensor(
                    out=ob[:, ib, :],
                    in0=pes,
                    scalar=rs[ib],
                    in1=bases[ib],
                    op0=MULT,
                    op1=ADD,
                )
                nc.vector.tensor_scalar_max(ob[:, ib, :], ob[:, ib, :], 0.0)

            # software-pipelined emission: stt b+1 ahead of glue b so DVE
            # never stalls on a glue chain waiting for ACT.
            emit_stt(0)
            emit_mid()
            emit_red(0)
            emit_stt(1)
            emit_glue(0)
            emit_red(1)
            if JACT3 > 0:
                # an early slice of block 3 rides ACT's mid-stream slack
                nc.scalar.dma_start(
                    out=ef_ts[3][:, 0:JACT3, :],
                    in_=ef_ap[3 * PB : 4 * PB, 0:JACT3, :],
                )
            emit_stt(2)
            emit_glue(1)
            emit_red(2)
            emit_stt(3)
            emit_glue(2)
            emit_red(3)
            emit_glue(3)

            # ---- out DMAs on the idle SP queue: blocks 0-2 as soon as
            # their relu lands, block 3's 32KB alone on the tail ----
            nc.sync.dma_start(
                out=_view(out_ap, [[OD, PB], [OD * PB, NBLK - 1], [1, OD]]),
                in_=ob[:, 0 : NBLK - 1, :],
            )
            nc.sync.dma_start(
                out=_view(out_ap, [[OD, PB], [1, OD]],
                          extra_offset=OD * PB * (NBLK - 1)),
                in_=ob[:, NBLK - 1, :],
            )

        if repeat == 1:
            for _ in range(unroll):
                emit_body()
        else:
            with tc.For_i(0, repeat, 1):
                for _ in range(unroll):
                    emit_body()

    nc.compile()
    return nc


_NC_CACHE = None


def _get_nc():
    global _NC_CACHE
    if _NC_CACHE is None:
        _NC_CACHE = build_bass()
    return _NC_CACHE


def make_in_maps(inputs):
    w = {
        k: np.ascontiguousarray(np.asarray(inputs[k], dtype=np.float32))
        for k in ("Wn_w", "Wn_b", "We_w", "We_b", "Ws_w", "Ws_b")
    }
    h = np.asarray(inputs["h"], dtype=np.float32)
    adj = np.asarray(inputs["adj"], dtype=np.int32)
    ef = np.asarray(inputs["edge_feat"], dtype=np.float32)
    in_maps = []
    for c in range(NCORES):
        m = dict(w)
        m["h"] = np.ascontiguousarray(h[c])
        m["adj"] = np.ascontiguousarray(adj[c])
        m["edge_feat"] = np.ascontiguousarray(ef[c])
        in_maps.append(m)
    return in_maps


def run(inputs, trace=False):
    """Run on hardware; returns (full_output, BassKernelResults)."""
    nc = _get_nc()
    res = run_bass_kernel_spmd(nc, make_in_maps(inputs), list(range(NCORES)), trace=trace)
    out = np.stack(
        [np.asarray(res.results[c]["out"]) for c in range(NCORES)], axis=0
    ).astype(np.float32)
    return out, res


def kernel(**inputs):
    out, _ = run(inputs)
    return out
